# revision 29
# baseline (speedup 1.0000x reference)
"""Trainium2 Bass kernel for nn_FocalToVoxelNeXtBridge.

Pipeline (per NeuronCore, 8 cores = batch(2) x y-strip(4)):
  1. proj:   f = relu(X @ W'), BN1 folded into W' plus an appended ones-row
             (bias trick).  PE matmuls in bf16, PSUM f32.
  2. dedup:  duplicate voxels of one cell are laid out by the host in
             partition-aligned regions so cheap DVE adds fold rank>=1
             tokens into the rank-0 token (post-ReLU, matching reference
             semantics).
  3. scatter: ONE dma_scatter_add (SDMA CCE add) per y-band of the strip's
             dense BEV grid; all indices within a scatter are unique so the
             CCE read-modify-writes cannot race.  Pad tokens target trash
             rows appended to each band tensor.
  4. conv:   3x3 subm conv as 9 shifted bf16 matmuls per output row over
             dma_start_transpose-loaded dense^T rows, + a K=1 "penalty"
             matmul adding -1e30 at inactive cells (host knows the active
             mask from coords) so the final ReLU zeroes them.  BN2 scale is
             folded into conv weights, shift applied as per-partition ACT
             bias.  Output rows stored as (y, c, x); host transposes back.
  Conv groups are emitted interleaved with the per-band proj/scatter work so
  the PE overlaps both phases.
"""

import os

import numpy as np
import ml_dtypes

BF16 = ml_dtypes.bfloat16

B, Y, X, C, CIN = 2, 512, 512, 128, 192
N = 400000
EPS1, EPS2 = 1e-5, 1e-3
STRIPS = 4          # y-strips per batch entry
SH = Y // STRIPS    # 128 output rows per core
HLOC = SH + 2       # local dense rows incl. +-1 halo
BAND_ROWS = 10
NBANDS = HLOC // BAND_ROWS          # 13
BCELLS = BAND_ROWS * X              # 5120 cells per band (< int16 max)
NEG = -1e30

_PROG_CACHE: dict = {}
LAST_EXEC_NS = None
LAST_RESULTS = None


# ----------------------------------------------------------------- host plan

def _plan_core(bi, yi, xi, b, s):
    """Sorted voxel list for one core: by (band, cell); returns voxel ids,
    local cell, dup-rank, band."""
    y0 = s * SH
    lo = y0 - 1
    m = (bi == b) & (yi >= lo) & (yi <= y0 + SH)
    vox = np.nonzero(m)[0]
    cell = (yi[vox] - lo).astype(np.int64) * X + xi[vox]
    order = np.argsort(cell, kind="stable")
    vox, cell = vox[order], cell[order]
    first = np.r_[True, cell[1:] != cell[:-1]]
    runstart = np.maximum.accumulate(np.where(first, np.arange(len(cell)), 0))
    rank = np.arange(len(cell)) - runstart
    band = cell // BCELLS
    return vox, cell, rank, band


def _core_regions(vox, cell, rank, band):
    """Per band: (n_singles, [n_dup_region_r ...], voxel->(region, slot)).

    Regions per band: U (single-voxel cells, slot = cell order),
    D_r (rank-r tokens of multi-voxel cells, slot = cell position in
    (count desc, cell asc) order -- deeper regions are prefixes, so
    region r slot j is the same cell for every r).
    """
    out = []
    for j in range(NBANDS):
        m = band == j
        cj, rj, vj = cell[m], rank[m], vox[m]
        uniq, counts = np.unique(cj, return_counts=True)
        sing_mask = counts == 1
        n_u = int(sing_mask.sum())
        dup_idx = np.nonzero(~sing_mask)[0]
        dup_order = dup_idx[np.lexsort((uniq[dup_idx], -counts[dup_idx]))]
        slot_of_uniq = np.full(len(uniq), -1, np.int64)
        slot_of_uniq[dup_order] = np.arange(len(dup_order))
        sing_slot = np.cumsum(sing_mask) - 1
        ui = np.searchsorted(uniq, cj)
        is_single = sing_mask[ui]
        region = np.where(is_single, 0, rj + 1)   # 0 = U, 1 = D0, ...
        slot = np.where(is_single, sing_slot[ui], slot_of_uniq[ui])
        nreg = [int((counts > max(r, 1)).sum())
                for r in range(int(counts.max()) if len(counts) else 1)]
        out.append((n_u, nreg, vj, cj, region, slot))
    return out


# ------------------------------------------------------------- device program

def _build_program(capu, capd):
    import concourse.bacc as bacc
    import concourse.mybir as mybir
    import concourse.tile as tile

    dt = mybir.dt
    maxd = capd.shape[1]
    band_cap = capu + capd.sum(axis=1)           # tokens per band
    band_off = np.concatenate([[0], np.cumsum(band_cap)])[:-1]
    nsc = capu + capd[:, 0]                      # scattered tokens per band
    TOT = int(band_cap.sum())
    nc = bacc.Bacc("TRN2", target_bir_lowering=False, debug=False)

    h_xT = nc.dram_tensor("xT", [CIN + 1, TOT], dt.bfloat16, kind="ExternalInput")
    h_idx = nc.dram_tensor("idxw", [128, TOT // 16], dt.int16, kind="ExternalInput")
    h_w1 = nc.dram_tensor("w1", [CIN + 1, C], dt.bfloat16, kind="ExternalInput")
    h_cw = nc.dram_tensor("convw", [9, C, C], dt.bfloat16, kind="ExternalInput")
    h_ones = nc.dram_tensor("onesw", [1, C], dt.bfloat16, kind="ExternalInput")
    h_b2 = nc.dram_tensor("bias2", [C, 1], dt.float32, kind="ExternalInput")
    h_pen = nc.dram_tensor("pen", [1, SH * X], dt.bfloat16, kind="ExternalInput")
    h_out = nc.dram_tensor("out_t", [SH, C, X], dt.float32, kind="ExternalOutput")
    # +128 trash rows as the target for pad tokens (f=0 adds; never read back)
    dense = [
        nc.dram_tensor(f"dense{j}", [BCELLS + 128, C], dt.bfloat16)
        for j in range(NBANDS)
    ]

    with tile.TileContext(nc) as tc:
        with (
            tc.tile_pool(name="const", bufs=1) as wp,
            tc.tile_pool(name="xa", bufs=3) as xap,
            tc.tile_pool(name="xb", bufs=3) as xbp,
            tc.tile_pool(name="f", bufs=4) as fp,
            tc.tile_pool(name="rows", bufs=32) as rp,
            tc.tile_pool(name="osb", bufs=6) as op,
            tc.tile_pool(name="penp", bufs=3) as pnp,
            tc.tile_pool(name="pp", bufs=2, space="PSUM") as pp,
            tc.tile_pool(name="cp", bufs=3, space="PSUM") as cp,
        ):
            # ---- constants
            w1a = wp.tile([128, C], dt.bfloat16)
            w1b = wp.tile([CIN + 1 - 128, C], dt.bfloat16)
            nc.sync.dma_start(out=w1a[:], in_=h_w1[0:128, :])
            nc.sync.dma_start(out=w1b[:], in_=h_w1[128:, :])
            wconv = wp.tile([C, 9 * C], dt.bfloat16)
            for t in range(9):
                nc.sync.dma_start(out=wconv[:, C * t:C * (t + 1)], in_=h_cw[t])
            ones = wp.tile([1, C], dt.bfloat16)
            nc.sync.dma_start(out=ones[:], in_=h_ones[:])
            b2 = wp.tile([C, 1], dt.float32)
            nc.sync.dma_start(out=b2[:], in_=h_b2[:])
            idxs = wp.tile([128, TOT // 16], dt.int16)
            nc.sync.dma_start(out=idxs[:], in_=h_idx[:])
            zt = wp.tile([128, BCELLS // 2], dt.bfloat16)
            nc.vector.memset(zt[:], 0.0)
            for j in range(NBANDS):
                half = BCELLS // 2
                nc.sync.dma_start(out=dense[j][0:half, :], in_=zt[:])
                nc.sync.dma_start(out=dense[j][half:BCELLS, :], in_=zt[:])

            # ---- conv emission machinery (interleaved with bands)
            rows = [None] * HLOC

            def load_row(L):
                r = rp.tile([128, X], dt.bfloat16, tag="row", name=f"row{L}")
                j, lr = L // BAND_ROWS, L % BAND_ROWS
                nc.sync.dma_start_transpose(
                    out=r[:], in_=dense[j][lr * X:(lr + 1) * X, :])
                rows[L] = r

            TAPS = [(1, 1), (0, 1), (2, 1), (0, 0), (0, 2), (1, 0), (1, 2),
                    (2, 0), (2, 2)]

            def emit_group(g0):
                ys = range(g0, min(g0 + 4, SH))
                for y in ys:
                    assert rows[y + 2] is not None
                # one (128, 1024) PSUM tile per PAIR of rows (= 2 banks;
                # each row's 512-slice is exactly one bank, so start=True
                # bank-clear semantics stay per-row correct)
                pst = {p: cp.tile([128, 2 * X], dt.float32, tag="cps",
                                  name=f"cps{g0}_{p}") for p in (0, 1)}
                sl = {y: (pst[(y - g0) // 2], ((y - g0) % 2) * X) for y in ys}
                peng = pnp.tile([1, 4 * X], dt.bfloat16, tag="pen")
                nc.scalar.dma_start(out=peng[:, 0:len(ys) * X],
                                    in_=h_pen[0:1, g0 * X:(g0 + len(ys)) * X])
                for dy, dx in TAPS:
                    w = wconv[:, C * (dy * 3 + dx):C * (dy * 3 + dx + 1)]
                    for y in ys:
                        rhs = rows[y + dy]
                        t, o = sl[y]
                        if dx == 1:
                            nc.tensor.matmul(t[:, o:o + X], w, rhs[:, 0:X],
                                             start=(dy == 1), stop=False)
                        elif dx == 0:
                            nc.tensor.matmul(t[:, o + 1:o + X], w,
                                             rhs[:, 0:X - 1],
                                             start=False, stop=False)
                        else:
                            nc.tensor.matmul(t[:, o:o + X - 1], w, rhs[:, 1:X],
                                             start=False, stop=False)
                for y in ys:
                    t, o = sl[y]
                    nc.tensor.matmul(t[:, o:o + X], ones[:],
                                     peng[0:1, (y - g0) * X:(y - g0 + 1) * X],
                                     start=False, stop=(o == X))
                for p in (0, 1):
                    osb = op.tile([128, 2 * X], dt.float32, tag="osb",
                                  name=f"osb{g0}_{p}")
                    if p == 0:
                        nc.scalar.activation(
                            osb[:], pst[p][:],
                            mybir.ActivationFunctionType.Relu, bias=b2[:, 0:1])
                    else:
                        nc.vector.tensor_scalar(
                            out=osb[:], in0=pst[p][:], scalar1=b2[:, 0:1],
                            scalar2=0.0, op0=mybir.AluOpType.add,
                            op1=mybir.AluOpType.max)
                    y0, y1 = g0 + 2 * p, g0 + 2 * p + 1
                    nc.sync.dma_start(out=h_out[y0], in_=osb[:, 0:X])
                    nc.scalar.dma_start(out=h_out[y1], in_=osb[:, X:2 * X])

            next_g0 = [0]

            def emit_conv_up_to(g0_limit):
                while next_g0[0] < SH and next_g0[0] <= g0_limit:
                    emit_group(next_g0[0])
                    next_g0[0] += 4

            # ---- projection + fold + scatter, band by band
            xa_t, xb_t = {}, {}

            def load_band(j):
                cap = int(band_cap[j])
                c0 = int(band_off[j])
                xa_t[j] = xap.tile([128, cap], dt.bfloat16, tag="xa",
                                   name=f"xa{j}")
                xb_t[j] = xbp.tile([CIN + 1 - 128, cap], dt.bfloat16, tag="xb",
                                   name=f"xb{j}")
                nc.sync.dma_start(out=xa_t[j][:], in_=h_xT[0:128, c0:c0 + cap])
                nc.sync.dma_start(out=xb_t[j][:], in_=h_xT[128:, c0:c0 + cap])

            load_band(0)
            load_band(1)
            for j in range(NBANDS):
                if j + 2 < NBANDS:
                    load_band(j + 2)
                cap = int(band_cap[j])
                c0 = int(band_off[j])
                xa, xb = xa_t[j], xb_t[j]
                fb = fp.tile([128, cap], dt.bfloat16, tag="f")
                for g in range(0, cap, 512):
                    gw = min(512, cap - g)
                    ps = pp.tile([128, 512], dt.float32, tag="ps", name=f"ps{j}_{g}")
                    nt = gw // 128
                    for ti in range(nt):
                        o = g + ti * 128
                        nc.tensor.matmul(
                            ps[:, ti * 128:(ti + 1) * 128],
                            xa[:, o:o + 128], w1a[:],
                            start=(ti == 0), stop=False)
                    for ti in range(nt):
                        o = g + ti * 128
                        nc.tensor.matmul(
                            ps[:, ti * 128:(ti + 1) * 128],
                            xb[:, o:o + 128], w1b[:],
                            start=False, stop=(ti == nt - 1))
                    nc.vector.tensor_relu(out=fb[:, g:g + gw], in_=ps[:, 0:gw])
                # fold dup ranks r>=1 into the rank-0 region (slots are
                # partition-aligned: every region size is a multiple of 128)
                d0 = int(capu[j])
                off = d0 + int(capd[j, 0])
                for r in range(1, maxd):
                    w = int(capd[j, r])
                    if w == 0:
                        continue
                    nc.vector.tensor_add(out=fb[:, d0:d0 + w],
                                         in0=fb[:, d0:d0 + w],
                                         in1=fb[:, off:off + w])
                    off += w
                # single collision-free scatter for this band
                cr = int(nsc[j])
                src = fb[:, 0:cr].rearrange("p (t e) -> p t e", e=C)
                isl = idxs[:, c0 // 16:(c0 + cr) // 16]
                nc.gpsimd.dma_scatter_add(
                    dense[j][:, :], src, isl,
                    num_idxs=cr, num_idxs_reg=cr, elem_size=C)
                # conv rows up to band j-2 are final; emit their groups
                # BEFORE this band's row transposes so the out-DMAs are not
                # queued behind transposes stalled on this band's scatter
                emit_conv_up_to(10 * j - 26)
                for L in range(10 * j, 10 * j + BAND_ROWS):
                    load_row(L)

            emit_conv_up_to(SH)
    nc.finalize()
    return nc


# ------------------------------------------------------------------ execution

def _ensure_ntff_hook():
    """Profiling-only: rebuild the antenv.axon_hooks shim that bass_utils
    expects for trace=True under axon (absent in this image)."""
    import sys
    import types
    try:
        from antenv.axon_hooks import get_axon_ntff_profile_hook  # noqa: F401
        return
    except ImportError:
        pass
    try:
        import antenv
        from trn_agent_boot.trn_boot import _ntff_profile_via_ctypes
        mod = types.ModuleType("antenv.axon_hooks")
        state = {"h": None}
        mod.set_axon_ntff_profile_hook = lambda h: state.__setitem__("h", h)
        mod.get_axon_ntff_profile_hook = lambda: state["h"]
        sys.modules["antenv.axon_hooks"] = mod
        antenv.axon_hooks = mod
        mod.set_axon_ntff_profile_hook(
            _ntff_profile_via_ctypes("/opt/axon/libaxon_pjrt.so"))
    except Exception as e:  # pragma: no cover - profiling is best-effort
        print(f"ntff hook setup failed: {e}")


def kernel(**inputs):
    global LAST_EXEC_NS, LAST_RESULTS
    vf = np.asarray(inputs["voxel_features"], np.float32)
    vc = np.asarray(inputs["voxel_coords"], np.int32)
    W_proj = np.asarray(inputs["W_proj"], np.float32)
    b_proj = np.asarray(inputs["b_proj"], np.float32)
    g1 = np.asarray(inputs["bn1_gamma"], np.float32)
    be1 = np.asarray(inputs["bn1_beta"], np.float32)
    mu1 = np.asarray(inputs["bn1_mean"], np.float32)
    v1 = np.asarray(inputs["bn1_var"], np.float32)
    conv_w = np.asarray(inputs["conv_w"], np.float32)
    conv_b = np.asarray(inputs["conv_b"], np.float32)
    g2 = np.asarray(inputs["bn2_gamma"], np.float32)
    be2 = np.asarray(inputs["bn2_beta"], np.float32)
    mu2 = np.asarray(inputs["bn2_mean"], np.float32)
    v2 = np.asarray(inputs["bn2_var"], np.float32)

    s1 = g1 / np.sqrt(v1 + EPS1)
    t1 = (b_proj - mu1) * s1 + be1
    w1 = np.concatenate([W_proj * s1[None, :], t1[None, :]], 0)  # (193,128)
    s2 = g2 / np.sqrt(v2 + EPS2)
    t2 = (conv_b - mu2) * s2 + be2
    cw = (conv_w * s2[None, None, None, :]).reshape(9, C, C)

    bi, yi, xi = vc[:, 0], vc[:, 2], vc[:, 3]
    active = np.zeros((B, Y, X), bool)
    active[bi, yi, xi] = True

    plans = []
    maxd = 1
    for core in range(8):
        b, s = core // STRIPS, core % STRIPS
        regions = _core_regions(*_plan_core(bi, yi, xi, b, s))
        plans.append(regions)
        for n_u, nreg, *_ in regions:
            maxd = max(maxd, len(nreg))

    capu = np.zeros(NBANDS, np.int64)
    capd = np.zeros((NBANDS, maxd), np.int64)
    for regions in plans:
        for j, (n_u, nreg, *_rest) in enumerate(regions):
            capu[j] = max(capu[j], n_u)
            for r, n in enumerate(nreg):
                capd[j, r] = max(capd[j, r], n)
    capu = ((capu + 127) // 128) * 128
    capd = ((capd + 127) // 128) * 128
    band_cap = capu + capd.sum(axis=1)
    band_off = np.concatenate([[0], np.cumsum(band_cap)])[:-1]
    reg_off = []
    for j in range(NBANDS):
        offs = [0, int(capu[j])]
        for r in range(maxd - 1):
            offs.append(offs[-1] + int(capd[j, r]))
        reg_off.append(offs)          # region r starts at reg_off[j][r]
    TOT = int(band_cap.sum())

    in_maps = []
    onesw = np.ones((1, C), BF16)
    w1_b = w1.astype(BF16)
    cw_b = cw.astype(BF16)
    b2_h = t2.reshape(C, 1).astype(np.float32)
    for core in range(8):
        b, s = core // STRIPS, core % STRIPS
        xT = np.zeros((CIN + 1, TOT), BF16)
        idx = (BCELLS + (np.arange(TOT) % 128)).astype(np.int16)
        for j, (n_u, nreg, vj, cj, region, slot) in enumerate(plans[core]):
            tok = (band_off[j] + np.array(reg_off[j])[region] + slot
                   if len(vj) else np.zeros(0, np.int64))
            xT[:CIN, tok] = vf[vj].T.astype(BF16)
            xT[CIN, tok] = np.ones(len(vj), BF16)
            scat = region <= 1        # U and D0 carry the scatter index
            idx[tok[scat]] = (cj[scat] - j * BCELLS).astype(np.int16)
        idxw = np.tile(idx.reshape(TOT // 16, 16).T, (8, 1))  # (128, TOT/16)
        pena = np.where(active[b, s * SH:(s + 1) * SH], 0.0, NEG)
        pen = pena.reshape(1, SH * X).astype(BF16)
        in_maps.append(dict(
            xT=np.ascontiguousarray(xT),
            idxw=np.ascontiguousarray(idxw),
            w1=w1_b, convw=cw_b, onesw=onesw, bias2=b2_h,
            pen=np.ascontiguousarray(pen)))

    key = (TOT,) + tuple(capu.tolist()) + tuple(capd.flatten().tolist())
    if key not in _PROG_CACHE:
        _PROG_CACHE[key] = _build_program(capu, capd)
    nc = _PROG_CACHE[key]

    from concourse.bass_utils import run_bass_kernel_spmd
    trace = os.environ.get("KERNEL_TRACE", "0") == "1"
    if trace:
        _ensure_ntff_hook()
    res = run_bass_kernel_spmd(nc, in_maps, core_ids=list(range(8)), trace=trace)
    LAST_EXEC_NS = res.exec_time_ns
    LAST_RESULTS = res

    out = np.empty((B, Y, X, C), np.float32)
    for core in range(8):
        b, s = core // STRIPS, core % STRIPS
        r = res.results[core]["out_t"]  # (SH, C, X)
        out[b, s * SH:(s + 1) * SH] = r.transpose(0, 2, 1)
    return out
